# revision 26
# baseline (speedup 1.0000x reference)
"""Affinity-propagation depth completion on 8 trn2 NeuronCores.

Algorithm (per image): `times` iterations of a spatially-varying 3x3
convolution (per-pixel normalized |affinity| weights, sparse-depth
re-injection each step).

Sharding: pure data-parallel, one image (batch element) per NeuronCore.

Per-core layout:
  - Image rows are split into 128 strips of 3 rows (352 rows -> 384 padded).
    Partition p owns rows [3p, 3p+2]; pad rows stay zero.
  - Each row is stored padded to W2 = 1218 (zero cols 0 and 1217).
  - Feature tiles are [128, 5, 1218] fp16: one halo row above/below the
    3 owned rows, maintained each iteration, so ALL nine 3x3 taps become
    pure free-dim offsets.
  - The 9 affinity planes are pre-normalized, pre-multiplied by (1-mask),
    and pre-shifted by their dx so the 9 per-pixel products run on the DVE
    at fp16 2x mode with 4B-aligned operands.
  - Per iteration: DVE does 9 elementwise products; the TensorEngine sums
    them (plus the fp16 sparse-depth plane) into PSUM fp32 via
    identity-matmul accumulation over 8 one-bank chunks; ScalarE evacuates
    PSUM back to the fp16 feature buffer and refreshes halos.
"""

import numpy as np

import concourse.bacc as bacc
import concourse.bass as bass
import concourse.mybir as mybir
import concourse.tile as tile
from concourse.bass_utils import run_bass_kernel_spmd
from concourse.masks import make_identity

H, W = 352, 1216
W2 = W + 2                      # padded row width
RPP = 3                         # rows per partition
P = 128
FLAT = RPP * W2                 # 3654 elements per partition (3 owned rows)
MAIN_P = H // RPP               # 117 partitions hold 3 full rows
TAIL_ROWS = H - MAIN_P * RPP    # 1 row left on partition 117
FP16 = mybir.dt.float16
FP32 = mybir.dt.float32
N_CORES = 8
TAPS = [(k // 3 - 1, k % 3 - 1) for k in range(9)]  # (dy, dx) per plane

# PSUM chunks covering out flat positions [1 .. 3652] (skip the extreme
# pad elements so dx-shifted reads stay inside [0, FLAT)). One bank each.
# Used for the one-time |affinity| sum.
CHUNKS = [(1 + 512 * q, 512) for q in range(7)] + [(1 + 512 * 7, FLAT - 2 - 512 * 7)]

# Loop-phase PSUM chunks: (col0, width) column bands covering all 3 owned
# rows (3 x 152 = 456 fp32 <= one bank). Interior-only (cols [1, 1216]) so
# evacuation never touches the zero pad columns; 8 chunks = 8 banks.
COLCHUNKS = [(1 + 152 * c, 152) for c in range(8)]
# dy = 0 taps first: the next iteration's dy=0 products need no halo rows,
# giving the halo DMAs ~6 us to land off the critical path.
KORDER = [3, 4, 5, 0, 1, 2, 6, 7, 8]


def _flat(ap):
    """[P, r, w] tile view -> [P, r*w]."""
    return ap.rearrange("p r w -> p (r w)")


import os

# profiling knob for timing studies (never set in production):
#   "muls" -> loop runs only the 9 DVE products
#   "pe"   -> loop runs only matmul accumulation + evac + halos
#   "nohalo" -> full loop minus halo DMAs (wrong numerics, timing probe)
KPROF = os.environ.get("KPROF", "")
# which product planes run on GPSIMD instead of DVE (concurrent engines;
# fp16 tensor_tensor is single-port so they never contend on SBUF ports)
GP_PLANES = frozenset(
    int(x) for x in os.environ.get("KGP", "").split(",") if x != ""
)


def build_body(tc, out_ap, aff_ap, feat_ap, sd_ap, times: int):
    nc = tc.nc


    with (
        tc.tile_pool(name="constp", bufs=1) as constp,
        tc.tile_pool(name="affp", bufs=10) as affp,
        tc.tile_pool(name="workp", bufs=8) as workp,
        tc.tile_pool(name="f32p", bufs=2) as f32p,
        tc.tile_pool(name="psump", bufs=8, space="PSUM") as psump,
    ):
        ident = constp.tile([P, P], FP16, name="ident")
        make_identity(nc, ident)

        msparse = constp.tile([P, RPP, W2], FP16, name="msparse")
        f0 = constp.tile([P, 5, W2], FP16, name="f0")
        f1 = constp.tile([P, 5, W2], FP16, name="f1")
        z32 = constp.tile([P, 1, W2], FP32, name="z32")
        nc.gpsimd.memset(msparse, 0.0)
        nc.gpsimd.memset(f0, 0.0)
        nc.gpsimd.memset(f1, 0.0)
        nc.gpsimd.memset(z32, 0.0)

        def _zbcast(nparts, nrows):
            """Broadcast view of the zero plane: [nparts, nrows, W2]."""
            a = z32[0:nparts, 0, :]
            return bass.AP(
                tensor=a.tensor,
                offset=a.offset,
                ap=[a.ap[0], [0, nrows], a.ap[1]],
            )

        def load_plane(src2d, name):
            """DRAM [H, W] fp32 -> fp32 scratch tile [P, RPP, W2] with the
            image interior at cols [1, 1216] and zeroed pad rows (partition
            117 rows 1-2, partitions 118-127). Pad cols stay garbage —
            callers read cols [1, 1216] only."""
            ld = f32p.tile([P, RPP, W2], FP32, name=name, tag="f32s")
            main = src2d[0 : MAIN_P * RPP, :].rearrange("(p r) w -> p r w", r=RPP)
            nc.sync.dma_start(out=ld[0:MAIN_P, :, 1 : 1 + W], in_=main)
            tail = src2d[MAIN_P * RPP : H, :].rearrange("(p r) w -> p r w", r=TAIL_ROWS)
            nc.sync.dma_start(
                out=ld[MAIN_P : MAIN_P + 1, 0:TAIL_ROWS, 1 : 1 + W], in_=tail
            )
            # zero the rows the DMA above does not cover (SBUF->SBUF DMA:
            # compute engines cannot start at partition 117)
            nc.sync.dma_start(
                out=ld[MAIN_P : MAIN_P + 1, TAIL_ROWS:RPP, :],
                in_=_zbcast(1, RPP - TAIL_ROWS),
            )
            nc.sync.dma_start(
                out=ld[MAIN_P + 1 : P, :, :], in_=_zbcast(P - MAIN_P - 1, RPP)
            )
            return ld

        # ---- load affinity planes: raw_k = |a_k| stored pre-shifted by dx ----
        raws = []
        for k, (dy, dx) in enumerate(TAPS):
            raw = affp.tile([P, RPP, W2], FP16, name=f"raw{k}", tag="aff")
            nc.gpsimd.memset(raw, 0.0)
            ld = load_plane(aff_ap[k], f"ld_a{k}")
            lo = 1 + dx
            nc.scalar.activation(
                out=raw[:, :, lo : lo + W],
                in_=ld[:, :, 1 : 1 + W],
                func=mybir.ActivationFunctionType.Abs,
            )
            raws.append(raw)

        # ---- sparse depth and feature ----
        ld_sd = load_plane(sd_ap, "ld_sd")
        nc.vector.tensor_copy(
            out=msparse[:, :, 1 : 1 + W], in_=ld_sd[:, :, 1 : 1 + W]
        )
        ld_ft = load_plane(feat_ap, "ld_ft")
        nc.vector.tensor_copy(
            out=f0[:, 1:4, 1 : 1 + W], in_=ld_ft[:, :, 1 : 1 + W]
        )

        msp_flat = _flat(msparse)
        raw_flats = [_flat(r) for r in raws]

        # ---- S = sum_k |a_k| via PE identity-matmul accumulation ----
        S = f32p.tile([P, RPP, W2], FP32, name="S", tag="f32s")
        nc.gpsimd.memset(S, 0.0)
        S_flat = _flat(S)
        psS = [psump.tile([P, 512], FP32, name=f"psS{c}", tag="ps") for c in range(8)]
        for k, (dy, dx) in enumerate(TAPS):
            for c, (s, ln) in enumerate(CHUNKS):
                nc.tensor.matmul(
                    psS[c][:, :ln],
                    ident,
                    raw_flats[k][:, s + dx : s + dx + ln],
                    start=(k == 0),
                    stop=(k == 8),
                )
        for c, (s, ln) in enumerate(CHUNKS):
            nc.scalar.copy(out=S_flat[:, s : s + ln], in_=psS[c][:, :ln])

        # ---- rinv = (1 - mask) / S, stored fp16, in 3 dx-shifted copies ----
        # eps keeps 1/S finite in fp16 at pad positions where S == 0
        # (1/1e-4 = 1e4 < 65504); real pixels have S = sum of 9 |normal|.
        nc.vector.tensor_scalar_max(out=S_flat, in0=S_flat, scalar1=1e-4)
        R = f32p.tile([P, RPP, W2], FP32, name="R", tag="f32s")
        nc.vector.reciprocal_approx_fast(out=_flat(R), in_=S_flat)
        inv = workp.tile([P, FLAT], FP16, name="inv", tag="work")
        nc.vector.tensor_scalar(
            out=inv, in0=msp_flat, scalar1=0.0, scalar2=None,
            op0=mybir.AluOpType.is_le,
        )
        rinv0 = workp.tile([P, FLAT], FP16, name="rinv0", tag="work")
        nc.vector.tensor_mul(rinv0, _flat(R), inv)
        rinvm = workp.tile([P, FLAT], FP16, name="rinvm", tag="work")
        rinvp = workp.tile([P, FLAT], FP16, name="rinvp", tag="work")
        nc.gpsimd.memset(rinvm, 0.0)
        nc.gpsimd.memset(rinvp, 0.0)
        nc.scalar.copy(out=rinvm[:, 0 : FLAT - 1], in_=rinv0[:, 1:FLAT])
        nc.scalar.copy(out=rinvp[:, 1:FLAT], in_=rinv0[:, 0 : FLAT - 1])
        rinv_by_dx = {-1: rinvm, 0: rinv0, 1: rinvp}

        # ---- aff_pre_k = |a_k| (shifted) * rinv (same shift) ----
        pres = []
        for k, (dy, dx) in enumerate(TAPS):
            pre = affp.tile([P, RPP, W2], FP16, name=f"pre{k}", tag="aff")
            nc.vector.tensor_mul(_flat(pre), raw_flats[k], rinv_by_dx[dx])
            pres.append(pre)

        # ---- f0 = (1-mask)*feature + sparse  (interior rows) ----
        f0_own = _flat(f0[:, 1:4, :])
        tmp = workp.tile([P, FLAT], FP16, name="tmp", tag="work")
        nc.vector.tensor_mul(tmp, f0_own, inv)
        nc.vector.tensor_add(f0_own, tmp, msp_flat)
        # halos for iteration 0 (partition-shifted copies must be DMA)
        nc.sync.dma_start(out=f0[1:P, 0, 1 : 1 + W], in_=f0[0 : P - 1, 3, 1 : 1 + W])
        nc.sync.dma_start(out=f0[0 : P - 1, 4, 1 : 1 + W], in_=f0[1:P, 1, 1 : 1 + W])

        # ---- main iteration loop ----
        out_f32 = None
        for t in range(times):
            cur, nxt = (f0, f1) if t % 2 == 0 else (f1, f0)
            last = t == times - 1

            if KPROF == "pe":
                prods = {k: pres[k] for k in range(9)}  # static, no mul dep
            else:
                prods = {}
                for k in KORDER:
                    dy, dx = TAPS[k]
                    p_k = workp.tile([P, RPP, W2], FP16, name=f"p{t}_{k}", tag="work")
                    eng = nc.gpsimd if k in GP_PLANES else nc.vector
                    eng.tensor_mul(p_k, pres[k], cur[:, 1 + dy : 4 + dy, :])
                    prods[k] = p_k

            if KPROF == "muls":
                continue

            ps = {}
            for c0, w in COLCHUNKS:
                pc = psump.tile([P, 512], FP32, name=f"ps{t}_{c0}", tag="ps")
                ps[c0] = pc
                nc.tensor.matmul(
                    pc[:, : RPP * w], ident, msparse[:, :, c0 : c0 + w],
                    start=True, stop=False,
                )
            for k in KORDER:
                dy, dx = TAPS[k]
                stop = k == KORDER[-1]
                for c0, w in COLCHUNKS:
                    nc.tensor.matmul(
                        ps[c0][:, : RPP * w],
                        ident,
                        prods[k][:, :, c0 + dx : c0 + dx + w],
                        start=False,
                        stop=stop,
                    )

            if last:
                out_f32 = f32p.tile([P, RPP, W2], FP32, name="outf32", tag="f32s")
                dst3 = out_f32
            else:
                dst3 = nxt[:, 1:4, :]
            for c0, w in COLCHUNKS:
                nc.scalar.copy(
                    out=dst3[:, :, c0 : c0 + w],
                    in_=ps[c0][:, : RPP * w].rearrange("p (r w) -> p r w", r=RPP),
                )

            if not last and KPROF != "nohalo":
                nc.sync.dma_start(
                    out=nxt[1:P, 0, 1 : 1 + W], in_=nxt[0 : P - 1, 3, 1 : 1 + W]
                )
                nc.sync.dma_start(
                    out=nxt[0 : P - 1, 4, 1 : 1 + W], in_=nxt[1:P, 1, 1 : 1 + W]
                )

        # ---- store the final fp32 result ----
        if out_f32 is None:  # KPROF timing modes only
            out_f32 = f32p.tile([P, RPP, W2], FP32, name="outf32", tag="f32s")
            nc.vector.memset(out_f32, 0.0)
        assert out_f32 is not None
        out_main = out_ap[0 : MAIN_P * RPP, :].rearrange("(p r) w -> p r w", r=RPP)
        nc.sync.dma_start(out=out_main, in_=out_f32[0:MAIN_P, :, 1 : 1 + W])
        out_tail = out_ap[MAIN_P * RPP : H, :].rearrange(
            "(p r) w -> p r w", r=TAIL_ROWS
        )
        nc.sync.dma_start(
            out=out_tail, in_=out_f32[MAIN_P : MAIN_P + 1, 0:TAIL_ROWS, 1 : 1 + W]
        )


def build(times: int) -> bass.Bass:
    nc = bacc.Bacc("TRN2", target_bir_lowering=False)
    aff = nc.dram_tensor("affinity", [9, H, W], FP32, kind="ExternalInput").ap()
    feat = nc.dram_tensor("feature", [H, W], FP32, kind="ExternalInput").ap()
    sd = nc.dram_tensor("sparse", [H, W], FP32, kind="ExternalInput").ap()
    out = nc.dram_tensor("out", [H, W], FP32, kind="ExternalOutput").ap()
    with tile.TileContext(nc) as tc:
        build_body(tc, out, aff, feat, sd, times)
    nc.compile()
    return nc


def kernel(**inputs) -> np.ndarray:
    affinity = np.ascontiguousarray(np.asarray(inputs["affinity"], dtype=np.float32))
    feature = np.ascontiguousarray(np.asarray(inputs["feature"], dtype=np.float32))
    sparse_depth = np.ascontiguousarray(
        np.asarray(inputs["sparse_depth"], dtype=np.float32)
    )
    times = int(np.asarray(inputs["times"]))
    B = affinity.shape[0]
    assert B == N_CORES and affinity.shape == (B, 9, H, W)

    nc = build(times)
    in_maps = [
        {
            "affinity": affinity[b],
            "feature": feature[b, 0],
            "sparse": sparse_depth[b, 0],
        }
        for b in range(B)
    ]
    res = run_bass_kernel_spmd(nc, in_maps, core_ids=list(range(N_CORES)))
    out = np.stack([r["out"] for r in res.results])[:, None]  # [B,1,H,W]
    return out.astype(np.float32)


if __name__ == "__main__":
    rng = np.random.default_rng(0)
    inputs = {
        "affinity": rng.standard_normal((8, 9, H, W), dtype=np.float32),
        "feature": rng.standard_normal((8, 1, H, W), dtype=np.float32),
        "sparse_depth": np.where(
            rng.uniform(size=(8, 1, H, W)) < 0.05,
            rng.uniform(0.1, 80.0, size=(8, 1, H, W)),
            0.0,
        ).astype(np.float32),
        "times": 24,
    }
    out = kernel(**inputs)
    print(out.shape, out.dtype)


# revision 27
# speedup vs baseline: 1.2807x; 1.2807x over previous
"""Affinity-propagation depth completion on 8 trn2 NeuronCores.

Algorithm (per image): `times` iterations of a spatially-varying 3x3
convolution (per-pixel normalized |affinity| weights, sparse-depth
re-injection each step).

Sharding: pure data-parallel, one image (batch element) per NeuronCore.

Per-core layout:
  - Image rows are split into 128 strips of 3 rows (352 rows -> 384 padded).
    Partition p owns rows [3p, 3p+2]; pad rows stay zero.
  - Each row is stored padded to W2 = 1218 (zero cols 0 and 1217).
  - Feature tiles are [128, 5, 1218] fp16: one halo row above/below the
    3 owned rows, maintained each iteration, so ALL nine 3x3 taps become
    pure free-dim offsets.
  - The 9 affinity planes are pre-normalized, pre-multiplied by (1-mask),
    and pre-shifted by their dx so the 9 per-pixel products run on the DVE
    at fp16 2x mode with 4B-aligned operands.
  - Per iteration: DVE does 9 elementwise products; the TensorEngine sums
    them (plus the fp16 sparse-depth plane) into PSUM fp32 via
    identity-matmul accumulation over 8 one-bank chunks; ScalarE evacuates
    PSUM back to the fp16 feature buffer and refreshes halos.
"""

import numpy as np

import concourse.bacc as bacc
import concourse.bass as bass
import concourse.mybir as mybir
import concourse.tile as tile
from concourse.bass_utils import run_bass_kernel_spmd
from concourse.masks import make_identity

H, W = 352, 1216
W2 = W + 2                      # padded row width
RPP = 3                         # rows per partition
P = 128
FLAT = RPP * W2                 # 3654 elements per partition (3 owned rows)
MAIN_P = H // RPP               # 117 partitions hold 3 full rows
TAIL_ROWS = H - MAIN_P * RPP    # 1 row left on partition 117
FP16 = mybir.dt.float16
FP32 = mybir.dt.float32
N_CORES = 8
TAPS = [(k // 3 - 1, k % 3 - 1) for k in range(9)]  # (dy, dx) per plane

# PSUM chunks covering out flat positions [1 .. 3652] (skip the extreme
# pad elements so dx-shifted reads stay inside [0, FLAT)). One bank each.
# Used for the one-time |affinity| sum.
CHUNKS = [(1 + 512 * q, 512) for q in range(7)] + [(1 + 512 * 7, FLAT - 2 - 512 * 7)]

# Loop-phase PSUM chunks: (col0, width) column bands covering all 3 owned
# rows (3 x 152 = 456 fp32 <= one bank). Interior-only (cols [1, 1216]) so
# evacuation never touches the zero pad columns; 8 chunks = 8 banks.
COLCHUNKS = [(1 + 152 * c, 152) for c in range(8)]
# dy = 0 taps first: the next iteration's dy=0 products need no halo rows,
# giving the halo DMAs ~6 us to land off the critical path.
KORDER = [3, 4, 5, 0, 1, 2, 6, 7, 8]


def _flat(ap):
    """[P, r, w] tile view -> [P, r*w]."""
    return ap.rearrange("p r w -> p (r w)")


import os

# profiling knob for timing studies (never set in production):
#   "muls" -> loop runs only the 9 DVE products
#   "pe"   -> loop runs only matmul accumulation + evac + halos
#   "nohalo" -> full loop minus halo DMAs (wrong numerics, timing probe)
KPROF = os.environ.get("KPROF", "")
# which product planes run on GPSIMD instead of DVE (concurrent engines;
# fp16 tensor_tensor is single-port so they never contend on SBUF ports)
GP_PLANES = frozenset(
    int(x) for x in os.environ.get("KGP", "").split(",") if x != ""
)


def build_body(tc, out_ap, aff_ap, feat_ap, sd_ap, times: int):
    nc = tc.nc


    with (
        tc.tile_pool(name="constp", bufs=1) as constp,
        tc.tile_pool(name="affp", bufs=10) as affp,
        tc.tile_pool(name="workp", bufs=8) as workp,
        tc.tile_pool(name="f32p", bufs=2) as f32p,
        tc.tile_pool(name="psump", bufs=8, space="PSUM") as psump,
    ):
        ident = constp.tile([P, P], FP16, name="ident")
        make_identity(nc, ident)

        msparse = constp.tile([P, RPP, W2], FP16, name="msparse")
        f0 = constp.tile([P, 5, W2], FP16, name="f0")
        f1 = constp.tile([P, 5, W2], FP16, name="f1")
        z32 = constp.tile([P, 1, W2], FP32, name="z32")
        nc.gpsimd.memset(msparse, 0.0)
        nc.gpsimd.memset(f0, 0.0)
        nc.gpsimd.memset(f1, 0.0)
        nc.gpsimd.memset(z32, 0.0)

        def _zbcast(nparts, nrows):
            """Broadcast view of the zero plane: [nparts, nrows, W2]."""
            a = z32[0:nparts, 0, :]
            return bass.AP(
                tensor=a.tensor,
                offset=a.offset,
                ap=[a.ap[0], [0, nrows], a.ap[1]],
            )

        def load_plane(src2d, name):
            """DRAM [H, W] fp32 -> fp32 scratch tile [P, RPP, W2] with the
            image interior at cols [1, 1216] and zeroed pad rows (partition
            117 rows 1-2, partitions 118-127). Pad cols stay garbage —
            callers read cols [1, 1216] only."""
            ld = f32p.tile([P, RPP, W2], FP32, name=name, tag="f32s")
            main = src2d[0 : MAIN_P * RPP, :].rearrange("(p r) w -> p r w", r=RPP)
            nc.sync.dma_start(out=ld[0:MAIN_P, :, 1 : 1 + W], in_=main)
            tail = src2d[MAIN_P * RPP : H, :].rearrange("(p r) w -> p r w", r=TAIL_ROWS)
            nc.sync.dma_start(
                out=ld[MAIN_P : MAIN_P + 1, 0:TAIL_ROWS, 1 : 1 + W], in_=tail
            )
            # zero the rows the DMA above does not cover (SBUF->SBUF DMA:
            # compute engines cannot start at partition 117)
            nc.sync.dma_start(
                out=ld[MAIN_P : MAIN_P + 1, TAIL_ROWS:RPP, :],
                in_=_zbcast(1, RPP - TAIL_ROWS),
            )
            nc.sync.dma_start(
                out=ld[MAIN_P + 1 : P, :, :], in_=_zbcast(P - MAIN_P - 1, RPP)
            )
            return ld

        # ---- load affinity planes: raw_k = |a_k| stored pre-shifted by dx ----
        raws = []
        for k, (dy, dx) in enumerate(TAPS):
            raw = affp.tile([P, RPP, W2], FP16, name=f"raw{k}", tag="aff")
            nc.gpsimd.memset(raw, 0.0)
            ld = load_plane(aff_ap[k], f"ld_a{k}")
            lo = 1 + dx
            nc.scalar.activation(
                out=raw[:, :, lo : lo + W],
                in_=ld[:, :, 1 : 1 + W],
                func=mybir.ActivationFunctionType.Abs,
            )
            raws.append(raw)

        # ---- sparse depth and feature ----
        ld_sd = load_plane(sd_ap, "ld_sd")
        nc.vector.tensor_copy(
            out=msparse[:, :, 1 : 1 + W], in_=ld_sd[:, :, 1 : 1 + W]
        )
        ld_ft = load_plane(feat_ap, "ld_ft")
        nc.vector.tensor_copy(
            out=f0[:, 1:4, 1 : 1 + W], in_=ld_ft[:, :, 1 : 1 + W]
        )

        msp_flat = _flat(msparse)
        raw_flats = [_flat(r) for r in raws]

        # mask complement and f0 = (1-mask)*feature + sparse: independent of
        # the affinity normalization chain, so do it while loads finish
        inv = workp.tile([P, FLAT], FP16, name="inv", tag="work")
        nc.vector.tensor_scalar(
            out=inv, in0=msp_flat, scalar1=0.0, scalar2=None,
            op0=mybir.AluOpType.is_le,
        )
        f0_own = _flat(f0[:, 1:4, :])
        tmp = workp.tile([P, FLAT], FP16, name="tmp", tag="work")
        nc.vector.tensor_mul(tmp, f0_own, inv)
        nc.vector.tensor_add(f0_own, tmp, msp_flat)
        nc.sync.dma_start(out=f0[1:P, 0, 1 : 1 + W], in_=f0[0 : P - 1, 3, 1 : 1 + W])
        nc.sync.dma_start(out=f0[0 : P - 1, 4, 1 : 1 + W], in_=f0[1:P, 1, 1 : 1 + W])

        # ---- S = sum_k |a_k| via PE identity-matmul accumulation ----
        S = f32p.tile([P, RPP, W2], FP32, name="S", tag="f32s")
        nc.gpsimd.memset(S, 0.0)
        S_flat = _flat(S)
        psS = [psump.tile([P, 512], FP32, name=f"psS{c}", tag="ps") for c in range(8)]
        for k, (dy, dx) in enumerate(TAPS):
            for c, (s, ln) in enumerate(CHUNKS):
                nc.tensor.matmul(
                    psS[c][:, :ln],
                    ident,
                    raw_flats[k][:, s + dx : s + dx + ln],
                    start=(k == 0),
                    stop=(k == 8),
                )
        for c, (s, ln) in enumerate(CHUNKS):
            nc.scalar.copy(out=S_flat[:, s : s + ln], in_=psS[c][:, :ln])

        # ---- rinv = (1 - mask) / S, stored fp16, in 3 dx-shifted copies ----
        # eps keeps 1/S finite in fp16 at pad positions where S == 0
        # (1/1e-4 = 1e4 < 65504); real pixels have S = sum of 9 |normal|.
        nc.vector.tensor_scalar_max(out=S_flat, in0=S_flat, scalar1=1e-4)
        R = f32p.tile([P, RPP, W2], FP32, name="R", tag="f32s")
        nc.vector.reciprocal_approx_fast(out=_flat(R), in_=S_flat)
        rinv0 = workp.tile([P, FLAT], FP16, name="rinv0", tag="work")
        nc.vector.tensor_mul(rinv0, _flat(R), inv)
        rinvm = workp.tile([P, FLAT], FP16, name="rinvm", tag="work")
        rinvp = workp.tile([P, FLAT], FP16, name="rinvp", tag="work")
        nc.gpsimd.memset(rinvm, 0.0)
        nc.gpsimd.memset(rinvp, 0.0)
        nc.vector.tensor_copy(out=rinvm[:, 0 : FLAT - 1], in_=rinv0[:, 1:FLAT])
        nc.scalar.copy(out=rinvp[:, 1:FLAT], in_=rinv0[:, 0 : FLAT - 1])
        rinv_by_dx = {-1: rinvm, 0: rinv0, 1: rinvp}

        # ---- aff_pre_k = |a_k| (shifted) * rinv (same shift) ----
        pres = []
        for k, (dy, dx) in enumerate(TAPS):
            pre = affp.tile([P, RPP, W2], FP16, name=f"pre{k}", tag="aff")
            nc.vector.tensor_mul(_flat(pre), raw_flats[k], rinv_by_dx[dx])
            pres.append(pre)

        # ---- main iteration loop ----
        out_f32 = None
        for t in range(times):
            cur, nxt = (f0, f1) if t % 2 == 0 else (f1, f0)
            last = t == times - 1

            if KPROF == "pe":
                prods = {k: pres[k] for k in range(9)}  # static, no mul dep
            else:
                prods = {}
                for k in KORDER:
                    dy, dx = TAPS[k]
                    p_k = workp.tile([P, RPP, W2], FP16, name=f"p{t}_{k}", tag="work")
                    eng = nc.gpsimd if k in GP_PLANES else nc.vector
                    eng.tensor_mul(p_k, pres[k], cur[:, 1 + dy : 4 + dy, :])
                    prods[k] = p_k

            if KPROF == "muls":
                continue

            ps = {}
            for c0, w in COLCHUNKS:
                pc = psump.tile([P, 512], FP32, name=f"ps{t}_{c0}", tag="ps")
                ps[c0] = pc
                nc.tensor.matmul(
                    pc[:, : RPP * w], ident, msparse[:, :, c0 : c0 + w],
                    start=True, stop=False,
                )
            for k in KORDER:
                dy, dx = TAPS[k]
                stop = k == KORDER[-1]
                for c0, w in COLCHUNKS:
                    nc.tensor.matmul(
                        ps[c0][:, : RPP * w],
                        ident,
                        prods[k][:, :, c0 + dx : c0 + dx + w],
                        start=False,
                        stop=stop,
                    )

            if last:
                out_f32 = f32p.tile([P, RPP, W2], FP32, name="outf32", tag="f32s")
                dst3 = out_f32
            else:
                dst3 = nxt[:, 1:4, :]
            for c0, w in COLCHUNKS:
                nc.scalar.copy(
                    out=dst3[:, :, c0 : c0 + w],
                    in_=ps[c0][:, : RPP * w].rearrange("p (r w) -> p r w", r=RPP),
                )

            if not last and KPROF != "nohalo":
                nc.sync.dma_start(
                    out=nxt[1:P, 0, 1 : 1 + W], in_=nxt[0 : P - 1, 3, 1 : 1 + W]
                )
                nc.sync.dma_start(
                    out=nxt[0 : P - 1, 4, 1 : 1 + W], in_=nxt[1:P, 1, 1 : 1 + W]
                )

        # ---- store the final fp32 result ----
        if out_f32 is None:  # KPROF timing modes only
            out_f32 = f32p.tile([P, RPP, W2], FP32, name="outf32", tag="f32s")
            nc.vector.memset(out_f32, 0.0)
        assert out_f32 is not None
        out_main = out_ap[0 : MAIN_P * RPP, :].rearrange("(p r) w -> p r w", r=RPP)
        nc.sync.dma_start(out=out_main, in_=out_f32[0:MAIN_P, :, 1 : 1 + W])
        out_tail = out_ap[MAIN_P * RPP : H, :].rearrange(
            "(p r) w -> p r w", r=TAIL_ROWS
        )
        nc.sync.dma_start(
            out=out_tail, in_=out_f32[MAIN_P : MAIN_P + 1, 0:TAIL_ROWS, 1 : 1 + W]
        )


def build(times: int) -> bass.Bass:
    nc = bacc.Bacc("TRN2", target_bir_lowering=False)
    aff = nc.dram_tensor("affinity", [9, H, W], FP32, kind="ExternalInput").ap()
    feat = nc.dram_tensor("feature", [H, W], FP32, kind="ExternalInput").ap()
    sd = nc.dram_tensor("sparse", [H, W], FP32, kind="ExternalInput").ap()
    out = nc.dram_tensor("out", [H, W], FP32, kind="ExternalOutput").ap()
    with tile.TileContext(nc) as tc:
        build_body(tc, out, aff, feat, sd, times)
    nc.compile()
    return nc


def kernel(**inputs) -> np.ndarray:
    affinity = np.ascontiguousarray(np.asarray(inputs["affinity"], dtype=np.float32))
    feature = np.ascontiguousarray(np.asarray(inputs["feature"], dtype=np.float32))
    sparse_depth = np.ascontiguousarray(
        np.asarray(inputs["sparse_depth"], dtype=np.float32)
    )
    times = int(np.asarray(inputs["times"]))
    B = affinity.shape[0]
    assert B == N_CORES and affinity.shape == (B, 9, H, W)

    nc = build(times)
    in_maps = [
        {
            "affinity": affinity[b],
            "feature": feature[b, 0],
            "sparse": sparse_depth[b, 0],
        }
        for b in range(B)
    ]
    res = run_bass_kernel_spmd(nc, in_maps, core_ids=list(range(N_CORES)))
    out = np.stack([r["out"] for r in res.results])[:, None]  # [B,1,H,W]
    return out.astype(np.float32)


if __name__ == "__main__":
    rng = np.random.default_rng(0)
    inputs = {
        "affinity": rng.standard_normal((8, 9, H, W), dtype=np.float32),
        "feature": rng.standard_normal((8, 1, H, W), dtype=np.float32),
        "sparse_depth": np.where(
            rng.uniform(size=(8, 1, H, W)) < 0.05,
            rng.uniform(0.1, 80.0, size=(8, 1, H, W)),
            0.0,
        ).astype(np.float32),
        "times": 24,
    }
    out = kernel(**inputs)
    print(out.shape, out.dtype)
